# revision 20
# baseline (speedup 1.0000x reference)
"""Trainium2 Bass kernel: multi-head self-attention with RoPE, causal mask.

Reference semantics (B=2, S=2048, D=1024, H=16, DK=64):
    q = rope(x @ Wq.T), k = rope(x @ Wk.T), v = x @ Wv.T   (per-head views)
    out = softmax(causal(q k^T / 8)) v ;  y = out @ Wo.T

Sharding over 8 cores: 2-way batch x 4-way heads (4 heads/core).
Each core computes a partial y [S, D] (its heads' contribution); host sums
the 4 partials per batch (device output is fp16, summed in fp64 on host).

On-device layout strategy (per core):
  - all 16-bit operands are fp16; host prepacks every weight/input so each
    SBUF tensor loads with ONE wide DMA (xT in 4 per-sg transfers); all
    loads ride the scalar queue in arrival-priority order (cos/sin last --
    the DVE-side rope needs them long after the PE needs x), sync carries
    only SBUF-SBUF swaps + y writebacks
  - stage A is emitted per-512-column group (sg): K ec0 + Q ec0 + four V
    chunks, so the PE streams as soon as each sg's xT slice lands; V is
    projected TRANSPOSED directly (lhsT = x chunk) -- no PE transpose
    pass; one strided copy scatters all 4 heads into the V working layout
    (ones col 0 for the softmax denominator, data at cols 64..127)
  - K lands naturally as [dk-pair, s] in ONE tile (h0 rows 0:64, h1 rows
    64:128); Q is written BLOCK-INTERLEAVED per 512-q-group with the other
    head's rows zeroed, so each non-diagonal score tile is a single
    1024-col matmul covering both heads (the zeros live on the Q side)
  - attention is HEAD-PAIR-OUTER with the ec1 K/Q projection chunks and
    the out-projection interleaved into the kc streams as PE filler;
    causal masks only touch the true 128-col diagonal sub-block
  - PSUM pools are shared across both stages (no release barrier between
    projection and attention)
  - normalization: reciprocal_approx_fast for both heads into one row,
    one f32r rounding copy, two accumulating PE matmuls broadcast into a
    single bc bank; multiply deferred into the next stream's slack
"""

import sys

sys.path.insert(0, "/opt/trn_rl_repo")

import numpy as np


S = 2048
D = 1024
NH = 16
DK = 64
HL = 4          # heads per core
EL = HL * DK    # 256 local e-dims
N_CORES = 8
THETA = 10000.0

_compiled = None


def _build():
    import concourse.bacc as bacc
    import concourse.tile as tile
    from concourse import mybir
    from concourse.alu_op_type import AluOpType

    dt = mybir.dt
    f32, f32r = dt.float32, dt.float32r
    f16 = dt.float16

    nc = bacc.Bacc("TRN2", target_bir_lowering=False, debug=False,
                   num_devices=N_CORES)

    xt_d = nc.dram_tensor("xt", [4, 128, 8, 512], f16, kind="ExternalInput").ap()
    wq_d = nc.dram_tensor("wq", [128, 8 * EL], f16, kind="ExternalInput").ap()
    wk_d = nc.dram_tensor("wk", [128, 8 * EL], f16, kind="ExternalInput").ap()
    wv_d = nc.dram_tensor("wv", [128, 8 * EL], f16, kind="ExternalInput").ap()
    wo_d = nc.dram_tensor("wo", [128, 2 * D], f16, kind="ExternalInput").ap()
    cos_d = nc.dram_tensor("cosT", [128, S], f16, kind="ExternalInput").ap()
    sin_d = nc.dram_tensor("sinT", [128, S], f16, kind="ExternalInput").ap()
    sel_d = nc.dram_tensor("sel2", [1, 256], f32r, kind="ExternalInput").ap()
    y = nc.dram_tensor("y", [S, D], f16, kind="ExternalOutput").ap()

    with tile.TileContext(nc) as tc:
        with tc.tile_pool(name="persist", bufs=1) as pp, \
             tc.tile_pool(name="kq", bufs=2) as kqp, \
             tc.tile_pool(name="rope", bufs=3) as ropp, \
             tc.tile_pool(name="ptp", bufs=8) as ptp, \
             tc.tile_pool(name="nrm", bufs=4) as nrmp, \
             tc.tile_pool(name="nrm2", bufs=2) as nrm2p, \
             tc.tile_pool(name="ysb", bufs=2) as ysbp, \
             tc.tile_pool(name="ps_s", bufs=2, space="PSUM") as ps_s, \
             tc.tile_pool(name="ps_pv", bufs=2, space="PSUM") as ps_pv, \
             tc.tile_pool(name="ps_y", bufs=1, space="PSUM") as ps_y, \
             tc.tile_pool(name="ps_bc", bufs=1, space="PSUM") as ps_bc:

            # persistent SBUF tiles (live across both stages)
            qt2 = [pp.tile([128, 2 * S], f16, tag=f"qt{c}", name=f"qt{c}") for c in range(2)]
            ktz = [pp.tile([128, S], f16, tag=f"ktz{c}", name=f"ktz{c}") for c in range(2)]
            vh_all = pp.tile([128, HL * 16 * 128], f16, tag="vh", name="vh")
            cos_sb = pp.tile([128, S], f16, tag="cos", name="cos")
            sin_sb = pp.tile([128, S], f16, tag="sin", name="sin")
            xt_all = pp.tile([128, 8 * S], f16, tag="xt", name="xt")
            wv_all = pp.tile([128, 8 * EL], f16, tag="wv", name="wv")
            wk_all = pp.tile([128, 8 * EL], f16, tag="wk", name="wk")
            wq_all = pp.tile([128, 8 * EL], f16, tag="wq", name="wq")
            wo_all = pp.tile([128, 2 * D], f16, tag="wo", name="wo")
            warm = pp.tile([128, 256], f16, tag="warm", name="warm")
            aot = [pp.tile([128, S], f16, tag=f"aot{c}", name=f"aot{c}") for c in range(2)]
            sel2 = pp.tile([1, 256], f32r, tag="sel2", name="sel2")

            xtv = xt_all[:].rearrange("p (d s) -> p d s", d=8)
            wvv = wv_all[:].rearrange("p (d e) -> p d e", d=8)
            wkv = wk_all[:].rearrange("p (d e) -> p d e", d=8)
            wqv = wq_all[:].rearrange("p (d e) -> p d e", d=8)
            wov = wo_all[:].rearrange("p (c d) -> p c d", c=2)
            vhv = vh_all[:].rearrange("p (h s c) -> p h s c", h=HL, c=128)
            qv = [qt2[c][:].rearrange("p (g h q) -> p g h q", h=2, q=512)
                  for c in range(2)]

            # ---- input DMA program ----
            nc.scalar.dma_start(wv_all[:], wv_d[:])
            nc.scalar.dma_start(xtv[:, :, 0:512], xt_d[0])
            nc.scalar.dma_start(wk_all[:], wk_d[:])
            nc.scalar.dma_start(xtv[:, :, 512:1024], xt_d[1])
            nc.scalar.dma_start(wq_all[:], wq_d[:])
            nc.scalar.dma_start(xtv[:, :, 1024:1536], xt_d[2])
            nc.scalar.dma_start(xtv[:, :, 1536:2048], xt_d[3])
            nc.scalar.dma_start(wo_all[:], wo_d[:])
            nc.scalar.dma_start(cos_sb[:], cos_d[:])
            nc.scalar.dma_start(sin_sb[:], sin_d[:])
            nc.scalar.dma_start(sel2[:], sel_d[:])

            # rope chunk: evacuate PSUM proj, SBUF-to-SBUF DMA block swap to
            # build the rotate-half partner, cos (DVE) / sin (gpsimd)
            # multiplies, adds into K or block-interleaved Q (DVE)
            def rope_chunk(ps, qa, qas, sg, is_k, ec, evac):
                sl = slice(512 * sg, 512 * (sg + 1))
                evac(qa[:, sl], ps[:])
                for blk in range(2):
                    b0 = 64 * blk
                    nc.sync.dma_start(
                        qas[b0:b0 + 32, sl], qa[b0 + 32:b0 + 64, sl])
                    nc.sync.dma_start(
                        qas[b0 + 32:b0 + 64, sl], qa[b0:b0 + 32, sl])
                qc = ropp.tile([128, 512], f16, tag="qc", name="qc")
                qs = ropp.tile([128, 512], f16, tag="qs", name="qs")
                nc.vector.tensor_mul(qc[:], qa[:, sl], cos_sb[:, sl])
                nc.gpsimd.tensor_mul(qs[:], qas[:, sl], sin_sb[:, sl])
                if is_k:
                    nc.vector.tensor_add(
                        ktz[ec][0:64, sl], qc[0:64, :], qs[0:64, :])
                    nc.vector.tensor_add(
                        ktz[ec][64:128, sl], qc[64:128, :], qs[64:128, :])
                else:
                    nc.vector.tensor_add(
                        qv[ec][0:64, sg, 0, :], qc[0:64, :], qs[0:64, :])
                    nc.vector.tensor_add(
                        qv[ec][64:128, sg, 1, :], qc[64:128, :], qs[64:128, :])

            # ======== stage A: per-sg V + K/Q ec0 projections ========
            # warm up the PE clock-gate while input DMAs land
            nc.vector.memset(warm[:], 0.0)
            wp = ps_y.tile([128, 512], f32, tag="yp", name="yp")
            for _ in range(26):
                nc.tensor.matmul(wp[:, 0:256], warm[:, 0:128], warm[:],
                                 start=True, stop=True)

            # zero the other-head rows of the block-interleaved Q + the
            # softmax-denominator ones column (gpsimd is idle here)
            for c in range(2):
                nc.gpsimd.memset(qv[c][64:128, :, 0, :], 0.0)
                nc.gpsimd.memset(qv[c][0:64, :, 1, :], 0.0)
            nc.gpsimd.memset(vhv[:, :, :, 0:1], 1.0)

            kq_qa = {}
            for is_k in (True, False):
                kq_qa[is_k] = (
                    kqp.tile([128, S], f16, tag="qa", name="qa"),
                    kqp.tile([128, S], f16, tag="qas", name="qas"))

            def kq_chunk(sg, is_k, wsv):
                sl = slice(512 * sg, 512 * (sg + 1))
                ps = ps_s.tile([128, 1024], f32, tag="ps", name="ps")
                for dc in range(8):
                    nc.tensor.matmul(
                        ps[:, 0:512], wsv[:, dc, 0:128], xtv[:, dc, sl],
                        start=(dc == 0), stop=(dc == 7))
                qa, qas = kq_qa[is_k]
                rope_chunk(ps[:, 0:512], qa, qas, sg, is_k, 0,
                           evac=nc.scalar.copy)

            def v_chunk(sc):
                psv = ps_pv.tile([128, 512], f32, tag="ppv", name="ppv")
                for dc in range(8):
                    nc.tensor.matmul(
                        psv[:, 0:256],
                        xtv[:, dc, 128 * sc:128 * (sc + 1)],
                        wvv[:, dc, :],
                        start=(dc == 0), stop=(dc == 7))
                eng = nc.scalar.copy if sc % 2 else nc.vector.tensor_copy
                eng(vhv[:, :, sc, 64:128],
                    psv[:, 0:256].rearrange("p (h e) -> p h e", h=HL))

            for sg in range(4):
                if sg == 0:
                    # sg0: V first -- wv+xt0 land before wk
                    for i in range(4):
                        v_chunk(i)
                    kq_chunk(0, True, wkv)
                    kq_chunk(0, False, wqv)
                else:
                    kq_chunk(sg, True, wkv)
                    kq_chunk(sg, False, wqv)
                    for i in range(4):
                        v_chunk(4 * sg + i)

            # ======== stage B: attention (head-pair outer) + out-proj ========
            # ec1 projection chunks, emitted as PE filler inside head-
            # pair 0's attention stream (use the out-projection's PSUM
            # slot, which is idle until head-pair 1)
            kq_tiles = {}
            for is_k in (True, False):
                kq_tiles[is_k] = (
                    kqp.tile([128, S], f16, tag="qa", name="qa"),
                    kqp.tile([128, S], f16, tag="qas", name="qas"))

            def mk_proj_pieces(is_k, sg):
                box = {}

                def piece1():
                    wsv = wkv if is_k else wqv
                    box["ps"] = ps_y.tile([128, 512], f32, tag="yp", name="yp")
                    for dc in range(4):
                        nc.tensor.matmul(
                            box["ps"][:],
                            wsv[:, dc, 128:256],
                            xtv[:, dc, 512 * sg:512 * (sg + 1)],
                            start=(dc == 0), stop=False)

                def piece2():
                    wsv = wkv if is_k else wqv
                    qa, qas = kq_tiles[is_k]
                    for dc in range(4, 8):
                        nc.tensor.matmul(
                            box["ps"][:],
                            wsv[:, dc, 128:256],
                            xtv[:, dc, 512 * sg:512 * (sg + 1)],
                            start=False, stop=(dc == 7))
                    rope_chunk(ps=box["ps"], qa=qa, qas=qas, sg=sg,
                               is_k=is_k, ec=1, evac=nc.vector.tensor_copy)
                return [piece1, piece2]

            filler_q = []
            for is_k in (True, False):
                for sg in range(4):
                    filler_q += mk_proj_pieces(is_k, sg)

            def out_proj_eg(sc, eg, ysb):
                yp = ps_y.tile([128, 512], f32, tag="yp", name="yp")
                for c2 in range(2):
                    nc.tensor.matmul(
                        yp[:],
                        aot[c2][:, 128 * sc:128 * (sc + 1)],
                        wov[:, c2, 512 * eg:512 * (eg + 1)],
                        start=(c2 == 0), stop=(c2 == 1))
                nc.vector.tensor_copy(
                    ysb[:, 512 * eg:512 * (eg + 1)], yp[:])
                if eg == 1:
                    for half in range(2):
                        sl = slice(512 * half, 512 * (half + 1))
                        nc.sync.dma_start(
                            y[128 * sc:128 * (sc + 1), sl], ysb[:, sl])

            def out_proj_pieces(sc):
                box = {}

                def p1():
                    box["ysb"] = ysbp.tile([128, D], f16, tag="ysb", name="ysb")
                    out_proj_eg(sc, 0, box["ysb"])

                def p2():
                    out_proj_eg(sc, 1, box["ysb"])
                return [p1, p2]

            pending = []   # deferred normalize closures

            def emit_pending_one():
                if pending:
                    pending.pop(0)()

            SKEW = 3
            for hp in range(2):
                for qg in range(4):
                    n_kc = 4 * qg + 4
                    # flush the previous stream's normalize early, then
                    # enqueue that q-group's out-projection pieces as
                    # per-kc PE filler
                    norm_at = {} if (hp, qg) == (0, 0) else {1: 1}
                    if hp == 1 and qg >= 1:
                        for sc in range(4 * (qg - 1), 4 * qg):
                            filler_q += out_proj_pieces(sc)
                    ppv = {}
                    for hh in range(2):
                        h = 2 * hp + hh
                        ppv[h] = ps_pv.tile([128, 512], f32, tag="ppv", name="ppv")
                    ptq = {}
                    for kc in range(n_kc + SKEW):
                        for _ in range(norm_at.get(kc, 0)):
                            emit_pending_one()
                        if kc >= 2 and filler_q:
                            filler_q.pop(0)()
                        # PV first: keeps queued work ahead of a score
                        # matmul that may block on PSUM reuse
                        kcp = kc - SKEW
                        if kcp >= 0:
                            ptv2, q0v = ptq.pop(kcp)
                            for hh in range(2):
                                h = 2 * hp + hh
                                nc.tensor.matmul(
                                    ppv[h][:, q0v:512],
                                    vhv[:, h, kcp, :],
                                    ptv2[:, 512 * hh + q0v:512 * (hh + 1)],
                                    start=(kcp == 0), stop=(kcp == n_kc - 1))
                        if kc < n_kc:
                            # diagonal tiles only need q >= k
                            r = kc - 4 * qg
                            q0 = 128 * r if r > 0 else 0
                            ps2 = ps_s.tile([128, 1024], f32, tag="ps", name="ps")
                            # per-head matmuls; the other head's rows are
                            # zero on the Q side (max matmul N is one bank)
                            for hh in range(2):
                                nc.tensor.matmul(
                                    ps2[:, 512 * hh + q0:512 * (hh + 1)],
                                    ktz[hp][:, 128 * kc:128 * (kc + 1)],
                                    qv[hp][:, qg, hh, q0:512],
                                    start=True, stop=True)
                            pt = ptp.tile([128, 1024], f16, tag="pt", name="pt")
                            psv2 = ps2[:].rearrange("p (h q) -> p h q", h=2)[:, :, q0:512]
                            ptv = pt[:].rearrange("p (h q) -> p h q", h=2)[:, :, q0:512]
                            nc.scalar.activation(
                                ptv, psv2,
                                mybir.ActivationFunctionType.Exp,
                                scale=0.125)
                            if r >= 0:
                                # only the 128-col diagonal sub-block can
                                # have q < k; the rest is already causal
                                for hh in range(2):
                                    nc.gpsimd.affine_select(
                                        pt[:, 512 * hh + q0:512 * hh + q0 + 128],
                                        pt[:, 512 * hh + q0:512 * hh + q0 + 128],
                                        pattern=[[1, 128]],
                                        compare_op=AluOpType.is_ge, fill=0.0,
                                        base=512 * qg + q0 - 128 * kc,
                                        channel_multiplier=-1)
                            ptq[kc] = (pt, q0)
                    # evacuate ppv fast: BOTH attn-out+denom copies first
                    # (they gate PSUM reuse), then the cheap reciprocals
                    daos = []
                    for hh in range(2):
                        h = 2 * hp + hh
                        dao = nrmp.tile([128, 512], f32, tag="dao", name="dao")
                        nc.vector.tensor_copy(dao[:], ppv[h][:])
                        daos.append(dao)
                    recf = nrm2p.tile([1, 1024], f32, tag="rec", name="rec")
                    for hh in range(2):
                        nc.vector.reciprocal_approx_fast(
                            recf[0:1, 512 * hh:512 * (hh + 1)],
                            daos[hh][0:1, :])
                    recr = nrm2p.tile([1, 1024], f32r, tag="recr", name="recr")
                    nc.vector.tensor_copy(recr[:], recf[:])

                    def mk_norm(qg=qg, c2=hp, rec=recr, daos=daos):
                        def emit():
                            # two accumulating matmuls broadcast BOTH
                            # heads' 1/denom into one bc bank, then
                            # normalize into aot
                            bc = ps_bc.tile([128, 512], f32, tag="bc", name="bc")
                            for hh in range(2):
                                nc.tensor.matmul(
                                    bc[:],
                                    sel2[0:1, 128 * hh:128 * (hh + 1)],
                                    rec[0:1, 512 * hh:512 * (hh + 1)],
                                    start=(hh == 0), stop=(hh == 1))
                            for hh in range(2):
                                nc.vector.tensor_mul(
                                    aot[c2][64 * hh:64 * hh + 64,
                                            512 * qg:512 * (qg + 1)],
                                    daos[hh][64:128, :],
                                    bc[64 * hh:64 * hh + 64, :])
                        return emit
                    pending.append(mk_norm())
            # tail: the remaining normalize, then the last four
            # out-projection chunks out of wide ps_s tiles; split the
            # evacuation across ACT + DVE and the writeback DMAs across
            # the sync + gpsimd queues.
            while pending:
                emit_pending_one()
            for i in range(4):
                sc = 12 + i
                ps2 = ps_s.tile([128, 1024], f32, tag="ps", name="ps")
                for eg in range(2):
                    for c2 in range(2):
                        nc.tensor.matmul(
                            ps2[:, 512 * eg:512 * (eg + 1)],
                            aot[c2][:, 128 * sc:128 * (sc + 1)],
                            wov[:, c2, 512 * eg:512 * (eg + 1)],
                            start=(c2 == 0), stop=(c2 == 1))
                ysb = ysbp.tile([128, D], f16, tag="ysb", name="ysb")
                nc.scalar.copy(ysb[:, 0:512], ps2[:, 0:512])
                nc.vector.tensor_copy(ysb[:, 512:1024], ps2[:, 512:1024])
                for half in range(2):
                    sl = slice(512 * half, 512 * (half + 1))
                    eng = nc.sync if half == 0 else nc.gpsimd
                    eng.dma_start(
                        y[128 * sc:128 * (sc + 1), sl], ysb[:, sl])

    nc.compile()
    return nc


def _prep_inputs(x, token_positions, Wq, Wk, Wv, Wo):
    # even/odd interleave permutation within each head (for rotate-half RoPE)
    perm = np.concatenate([np.arange(0, DK, 2), np.arange(1, DK, 2)])

    pos = np.asarray(token_positions).astype(np.float32)
    angles = THETA ** (-np.arange(32, dtype=np.float32) / 32.0)
    ang = pos[:, None] * angles[None, :]          # [S, 32]
    cos32 = np.cos(ang).T.astype(np.float32)      # [32, S]
    sin32 = np.sin(ang).T.astype(np.float32)
    cos128 = np.concatenate([cos32, cos32, cos32, cos32], axis=0)
    sin128 = np.concatenate([-sin32, sin32, -sin32, sin32], axis=0)
    cos128 = np.ascontiguousarray(cos128).astype(np.float16)
    sin128 = np.ascontiguousarray(sin128).astype(np.float16)

    Wq = np.asarray(Wq, dtype=np.float32)
    Wk = np.asarray(Wk, dtype=np.float32)
    Wv = np.asarray(Wv, dtype=np.float32)
    Wo = np.asarray(Wo, dtype=np.float32)
    x = np.asarray(x, dtype=np.float32)

    f16 = np.float16

    def pack_w(wT):
        # [1024 d, 256 e] -> [128 p, 8 dc, 256 e]
        return np.ascontiguousarray(
            wT.reshape(8, 128, EL).transpose(1, 0, 2).reshape(128, 8 * EL)
        ).astype(f16)

    sel2 = np.zeros((1, 256), dtype=np.float32)
    sel2[0, 0:64] = 1.0
    sel2[0, 192:256] = 1.0

    in_maps = []
    for c in range(N_CORES):
        b = c // 4
        h0 = (c % 4) * HL
        esl = slice(h0 * DK, (h0 + HL) * DK)
        wq_h = Wq[esl].reshape(HL, DK, D)[:, perm].reshape(EL, D)
        wk_h = Wk[esl].reshape(HL, DK, D)[:, perm].reshape(EL, D)
        wv_h = Wv[esl]
        xT = x[b].T  # [1024 d, 2048 s]
        xt_p = np.ascontiguousarray(
            xT.reshape(8, 128, 4, 512).transpose(2, 1, 0, 3)).astype(f16)
        woT = Wo[:, esl].T  # [256 e, 1024 d_out]
        wo_p = np.ascontiguousarray(
            woT.reshape(2, 128, D).transpose(1, 0, 2).reshape(128, 2 * D)
        ).astype(f16)
        in_maps.append({
            "xt": xt_p,
            "wq": pack_w(wq_h.T),
            "wk": pack_w(wk_h.T),
            "wv": pack_w(wv_h.T),
            "wo": wo_p,
            "cosT": cos128,
            "sinT": sin128,
            "sel2": sel2,
        })
    return in_maps


def kernel(x, token_positions, Wq, Wk, Wv, Wo, _trace=False):
    from concourse.bass_utils import run_bass_kernel_spmd

    global _compiled
    if _compiled is None:
        _compiled = _build()
    in_maps = _prep_inputs(x, token_positions, Wq, Wk, Wv, Wo)
    res = run_bass_kernel_spmd(_compiled, in_maps, list(range(N_CORES)),
                               trace=_trace)
    parts = [res.results[c]["y"].astype(np.float64) for c in range(N_CORES)]
    out = np.empty((2, S, D), dtype=np.float32)
    out[0] = (parts[0] + parts[1] + parts[2] + parts[3]).astype(np.float32)
    out[1] = (parts[4] + parts[5] + parts[6] + parts[7]).astype(np.float32)
    if _trace:
        return out, res
    return out


# revision 22
# speedup vs baseline: 1.0351x; 1.0351x over previous
"""Trainium2 Bass kernel: multi-head self-attention with RoPE, causal mask.

Reference semantics (B=2, S=2048, D=1024, H=16, DK=64):
    q = rope(x @ Wq.T), k = rope(x @ Wk.T), v = x @ Wv.T   (per-head views)
    out = softmax(causal(q k^T / 8)) v ;  y = out @ Wo.T

Sharding over 8 cores: 2-way batch x 4-way heads (4 heads/core).
Each core computes a partial y [S, D] (its heads' contribution); host sums
the 4 partials per batch (device output is fp16, summed in fp64 on host).

On-device layout strategy (per core):
  - all 16-bit operands are fp16; host prepacks every weight/input so each
    SBUF tensor loads with ONE wide DMA (xT in 4 per-sg transfers); all
    loads ride the scalar queue in arrival-priority order (cos/sin last --
    the DVE-side rope needs them long after the PE needs x), sync carries
    only SBUF-SBUF swaps + y writebacks
  - stage A is emitted per-512-column group (sg): K ec0 + Q ec0 + four V
    chunks, so the PE streams as soon as each sg's xT slice lands; V is
    projected TRANSPOSED directly (lhsT = x chunk) -- no PE transpose
    pass; one strided copy scatters all 4 heads into the V working layout
    (ones col 0 for the softmax denominator, data at cols 64..127)
  - K lands naturally as [dk-pair, s] in ONE tile (h0 rows 0:64, h1 rows
    64:128); Q is written BLOCK-INTERLEAVED per 512-q-group with the other
    head's rows zeroed, so each non-diagonal score tile is a single
    1024-col matmul covering both heads (the zeros live on the Q side)
  - attention is HEAD-PAIR-OUTER with the ec1 K/Q projection chunks and
    the out-projection interleaved into the kc streams as PE filler;
    causal masks only touch the true 128-col diagonal sub-block
  - PSUM pools are shared across both stages (no release barrier between
    projection and attention)
  - normalization: reciprocal_approx_fast for both heads into one row,
    one f32r rounding copy, two accumulating PE matmuls broadcast into a
    single bc bank; multiply deferred into the next stream's slack
"""

import sys

sys.path.insert(0, "/opt/trn_rl_repo")

import numpy as np


S = 2048
D = 1024
NH = 16
DK = 64
HL = 4          # heads per core
EL = HL * DK    # 256 local e-dims
N_CORES = 8
THETA = 10000.0

_compiled = None


def _build():
    import concourse.bacc as bacc
    import concourse.tile as tile
    from concourse import mybir
    from concourse.alu_op_type import AluOpType

    dt = mybir.dt
    f32, f32r = dt.float32, dt.float32r
    f16 = dt.float16

    nc = bacc.Bacc("TRN2", target_bir_lowering=False, debug=False,
                   num_devices=N_CORES)

    xt_d = nc.dram_tensor("xt", [4, 128, 8, 512], f16, kind="ExternalInput").ap()
    wq_d = nc.dram_tensor("wq", [128, 8 * EL], f16, kind="ExternalInput").ap()
    wk_d = nc.dram_tensor("wk", [128, 8 * EL], f16, kind="ExternalInput").ap()
    wv_d = nc.dram_tensor("wv", [128, 8 * EL], f16, kind="ExternalInput").ap()
    wo_d = nc.dram_tensor("wo", [128, 2 * D], f16, kind="ExternalInput").ap()
    cos_d = nc.dram_tensor("cosT", [128, S], f16, kind="ExternalInput").ap()
    sin_d = nc.dram_tensor("sinT", [128, S], f16, kind="ExternalInput").ap()
    sel_d = nc.dram_tensor("sel2", [1, 256], f32r, kind="ExternalInput").ap()
    y = nc.dram_tensor("y", [S, D], f16, kind="ExternalOutput").ap()

    with tile.TileContext(nc) as tc:
        with tc.tile_pool(name="persist", bufs=1) as pp, \
             tc.tile_pool(name="kq", bufs=2) as kqp, \
             tc.tile_pool(name="rope", bufs=3) as ropp, \
             tc.tile_pool(name="ptp", bufs=8) as ptp, \
             tc.tile_pool(name="nrm", bufs=4) as nrmp, \
             tc.tile_pool(name="nrm2", bufs=2) as nrm2p, \
             tc.tile_pool(name="ysb", bufs=2) as ysbp, \
             tc.tile_pool(name="ps_s", bufs=2, space="PSUM") as ps_s, \
             tc.tile_pool(name="ps_pv", bufs=2, space="PSUM") as ps_pv, \
             tc.tile_pool(name="ps_y", bufs=1, space="PSUM") as ps_y, \
             tc.tile_pool(name="ps_bc", bufs=1, space="PSUM") as ps_bc:

            # persistent SBUF tiles (live across both stages)
            qt2 = [pp.tile([128, 2 * S], f16, tag=f"qt{c}", name=f"qt{c}") for c in range(2)]
            ktz = [pp.tile([128, S], f16, tag=f"ktz{c}", name=f"ktz{c}") for c in range(2)]
            vh_all = pp.tile([128, HL * 16 * 128], f16, tag="vh", name="vh")
            cos_sb = pp.tile([128, S], f16, tag="cos", name="cos")
            sin_sb = pp.tile([128, S], f16, tag="sin", name="sin")
            xt_all = pp.tile([128, 8 * S], f16, tag="xt", name="xt")
            wv_all = pp.tile([128, 8 * EL], f16, tag="wv", name="wv")
            wk_all = pp.tile([128, 8 * EL], f16, tag="wk", name="wk")
            wq_all = pp.tile([128, 8 * EL], f16, tag="wq", name="wq")
            wo_all = pp.tile([128, 2 * D], f16, tag="wo", name="wo")
            warm = pp.tile([128, 256], f16, tag="warm", name="warm")
            aot = [pp.tile([128, S], f16, tag=f"aot{c}", name=f"aot{c}") for c in range(2)]
            sel2 = pp.tile([1, 256], f32r, tag="sel2", name="sel2")

            xtv = xt_all[:].rearrange("p (d s) -> p d s", d=8)
            wvv = wv_all[:].rearrange("p (d e) -> p d e", d=8)
            wkv = wk_all[:].rearrange("p (d e) -> p d e", d=8)
            wqv = wq_all[:].rearrange("p (d e) -> p d e", d=8)
            wov = wo_all[:].rearrange("p (c d) -> p c d", c=2)
            vhv = vh_all[:].rearrange("p (h s c) -> p h s c", h=HL, c=128)
            qv = [qt2[c][:].rearrange("p (g h q) -> p g h q", h=2, q=512)
                  for c in range(2)]

            # ---- input DMA program ----
            nc.scalar.dma_start(wv_all[:], wv_d[:])
            nc.scalar.dma_start(xtv[:, :, 0:512], xt_d[0])
            nc.scalar.dma_start(wk_all[:], wk_d[:])
            nc.scalar.dma_start(xtv[:, :, 512:1024], xt_d[1])
            nc.scalar.dma_start(wq_all[:], wq_d[:])
            nc.scalar.dma_start(cos_sb[:], cos_d[:])
            nc.scalar.dma_start(sin_sb[:], sin_d[:])
            nc.scalar.dma_start(xtv[:, :, 1024:1536], xt_d[2])
            nc.scalar.dma_start(xtv[:, :, 1536:2048], xt_d[3])
            nc.scalar.dma_start(wo_all[:], wo_d[:])
            nc.scalar.dma_start(sel2[:], sel_d[:])

            # rope chunk: evacuate PSUM proj, SBUF-to-SBUF DMA block swap to
            # build the rotate-half partner, cos (DVE) / sin (gpsimd)
            # multiplies, adds into K or block-interleaved Q (DVE)
            def rope_chunk(ps, qa, qas, sg, is_k, ec, evac):
                sl = slice(512 * sg, 512 * (sg + 1))
                evac(qa[:, sl], ps[:])
                for blk in range(2):
                    b0 = 64 * blk
                    nc.sync.dma_start(
                        qas[b0:b0 + 32, sl], qa[b0 + 32:b0 + 64, sl])
                    nc.sync.dma_start(
                        qas[b0 + 32:b0 + 64, sl], qa[b0:b0 + 32, sl])
                qc = ropp.tile([128, 512], f16, tag="qc", name="qc")
                qs = ropp.tile([128, 512], f16, tag="qs", name="qs")
                nc.vector.tensor_mul(qc[:], qa[:, sl], cos_sb[:, sl])
                nc.gpsimd.tensor_mul(qs[:], qas[:, sl], sin_sb[:, sl])
                if is_k:
                    nc.vector.tensor_add(
                        ktz[ec][0:64, sl], qc[0:64, :], qs[0:64, :])
                    nc.vector.tensor_add(
                        ktz[ec][64:128, sl], qc[64:128, :], qs[64:128, :])
                else:
                    nc.vector.tensor_add(
                        qv[ec][0:64, sg, 0, :], qc[0:64, :], qs[0:64, :])
                    nc.vector.tensor_add(
                        qv[ec][64:128, sg, 1, :], qc[64:128, :], qs[64:128, :])

            # ======== stage A: per-sg V + K/Q ec0 projections ========
            # warm up the PE clock-gate while input DMAs land
            nc.vector.memset(warm[:], 0.0)
            wp = ps_y.tile([128, 512], f32, tag="yp", name="yp")
            for _ in range(26):
                nc.tensor.matmul(wp[:, 0:256], warm[:, 0:128], warm[:],
                                 start=True, stop=True)

            # zero the other-head rows of the block-interleaved Q + the
            # softmax-denominator ones column (gpsimd is idle here)
            for c in range(2):
                nc.gpsimd.memset(qv[c][64:128, :, 0, :], 0.0)
                nc.gpsimd.memset(qv[c][0:64, :, 1, :], 0.0)
            nc.gpsimd.memset(vhv[:, :, :, 0:1], 1.0)

            kq_qa = {}
            for is_k in (True, False):
                kq_qa[is_k] = (
                    kqp.tile([128, S], f16, tag="qa", name="qa"),
                    kqp.tile([128, S], f16, tag="qas", name="qas"))

            def kq_chunk(sg, is_k, wsv):
                sl = slice(512 * sg, 512 * (sg + 1))
                ps = ps_s.tile([128, 1024], f32, tag="ps", name="ps")
                for dc in range(8):
                    nc.tensor.matmul(
                        ps[:, 0:512], wsv[:, dc, 0:128], xtv[:, dc, sl],
                        start=(dc == 0), stop=(dc == 7))
                qa, qas = kq_qa[is_k]
                rope_chunk(ps[:, 0:512], qa, qas, sg, is_k, 0,
                           evac=nc.scalar.copy)

            def v_chunk(sc):
                psv = ps_pv.tile([128, 512], f32, tag="ppv", name="ppv")
                for dc in range(8):
                    nc.tensor.matmul(
                        psv[:, 0:256],
                        xtv[:, dc, 128 * sc:128 * (sc + 1)],
                        wvv[:, dc, :],
                        start=(dc == 0), stop=(dc == 7))
                # always ACT: the DVE queue holds rope multiplies that wait
                # on cos/sin -- a DVE evac behind them would HOL-block the
                # PE on PSUM reuse
                nc.scalar.copy(vhv[:, :, sc, 64:128],
                               psv[:, 0:256].rearrange("p (h e) -> p h e", h=HL))

            for sg in range(4):
                if sg == 0:
                    # sg0: V first -- wv+xt0 land before wk
                    for i in range(4):
                        v_chunk(i)
                    kq_chunk(0, True, wkv)
                    kq_chunk(0, False, wqv)
                else:
                    kq_chunk(sg, True, wkv)
                    kq_chunk(sg, False, wqv)
                    for i in range(4):
                        v_chunk(4 * sg + i)

            # ======== stage B: attention (head-pair outer) + out-proj ========
            # ec1 projection chunks, emitted as PE filler inside head-
            # pair 0's attention stream (use the out-projection's PSUM
            # slot, which is idle until head-pair 1)
            kq_tiles = {}
            for is_k in (True, False):
                kq_tiles[is_k] = (
                    kqp.tile([128, S], f16, tag="qa", name="qa"),
                    kqp.tile([128, S], f16, tag="qas", name="qas"))

            def mk_proj_pieces(is_k, sg):
                box = {}

                def piece1():
                    wsv = wkv if is_k else wqv
                    box["ps"] = ps_y.tile([128, 512], f32, tag="yp", name="yp")
                    for dc in range(4):
                        nc.tensor.matmul(
                            box["ps"][:],
                            wsv[:, dc, 128:256],
                            xtv[:, dc, 512 * sg:512 * (sg + 1)],
                            start=(dc == 0), stop=False)

                def piece2():
                    wsv = wkv if is_k else wqv
                    qa, qas = kq_tiles[is_k]
                    for dc in range(4, 8):
                        nc.tensor.matmul(
                            box["ps"][:],
                            wsv[:, dc, 128:256],
                            xtv[:, dc, 512 * sg:512 * (sg + 1)],
                            start=False, stop=(dc == 7))
                    rope_chunk(ps=box["ps"], qa=qa, qas=qas, sg=sg,
                               is_k=is_k, ec=1, evac=nc.vector.tensor_copy)
                return [piece1, piece2]

            filler_q = []
            for is_k in (True, False):
                for sg in range(4):
                    filler_q += mk_proj_pieces(is_k, sg)

            def out_proj_eg(sc, eg, ysb):
                yp = ps_y.tile([128, 512], f32, tag="yp", name="yp")
                for c2 in range(2):
                    nc.tensor.matmul(
                        yp[:],
                        aot[c2][:, 128 * sc:128 * (sc + 1)],
                        wov[:, c2, 512 * eg:512 * (eg + 1)],
                        start=(c2 == 0), stop=(c2 == 1))
                nc.vector.tensor_copy(
                    ysb[:, 512 * eg:512 * (eg + 1)], yp[:])
                if eg == 1:
                    for half in range(2):
                        sl = slice(512 * half, 512 * (half + 1))
                        nc.sync.dma_start(
                            y[128 * sc:128 * (sc + 1), sl], ysb[:, sl])

            def out_proj_pieces(sc):
                box = {}

                def p1():
                    box["ysb"] = ysbp.tile([128, D], f16, tag="ysb", name="ysb")
                    out_proj_eg(sc, 0, box["ysb"])

                def p2():
                    out_proj_eg(sc, 1, box["ysb"])
                return [p1, p2]

            pending = []   # deferred normalize closures

            def emit_pending_one():
                if pending:
                    pending.pop(0)()

            SKEW = 3
            for hp in range(2):
                for qg in range(4):
                    n_kc = 4 * qg + 4
                    # flush the previous stream's normalize early, then
                    # enqueue that q-group's out-projection pieces as
                    # per-kc PE filler
                    norm_at = {} if (hp, qg) == (0, 0) else {1: 1}
                    if hp == 1 and qg >= 1:
                        for sc in range(4 * (qg - 1), 4 * qg):
                            filler_q += out_proj_pieces(sc)
                    ppv = {}
                    for hh in range(2):
                        h = 2 * hp + hh
                        ppv[h] = ps_pv.tile([128, 512], f32, tag="ppv", name="ppv")
                    ptq = {}
                    for kc in range(n_kc + SKEW):
                        for _ in range(norm_at.get(kc, 0)):
                            emit_pending_one()
                        if kc >= 2 and filler_q:
                            filler_q.pop(0)()
                        # PV first: keeps queued work ahead of a score
                        # matmul that may block on PSUM reuse
                        kcp = kc - SKEW
                        if kcp >= 0:
                            ptv2, q0v = ptq.pop(kcp)
                            for hh in range(2):
                                h = 2 * hp + hh
                                nc.tensor.matmul(
                                    ppv[h][:, q0v:512],
                                    vhv[:, h, kcp, :],
                                    ptv2[:, 512 * hh + q0v:512 * (hh + 1)],
                                    start=(kcp == 0), stop=(kcp == n_kc - 1))
                        if kc < n_kc:
                            # diagonal tiles only need q >= k
                            r = kc - 4 * qg
                            q0 = 128 * r if r > 0 else 0
                            ps2 = ps_s.tile([128, 1024], f32, tag="ps", name="ps")
                            # per-head matmuls; the other head's rows are
                            # zero on the Q side (max matmul N is one bank)
                            for hh in range(2):
                                nc.tensor.matmul(
                                    ps2[:, 512 * hh + q0:512 * (hh + 1)],
                                    ktz[hp][:, 128 * kc:128 * (kc + 1)],
                                    qv[hp][:, qg, hh, q0:512],
                                    start=True, stop=True)
                            pt = ptp.tile([128, 1024], f16, tag="pt", name="pt")
                            psv2 = ps2[:].rearrange("p (h q) -> p h q", h=2)[:, :, q0:512]
                            ptv = pt[:].rearrange("p (h q) -> p h q", h=2)[:, :, q0:512]
                            nc.scalar.activation(
                                ptv, psv2,
                                mybir.ActivationFunctionType.Exp,
                                scale=0.125)
                            if r >= 0:
                                # only the 128-col diagonal sub-block can
                                # have q < k; the rest is already causal
                                for hh in range(2):
                                    nc.gpsimd.affine_select(
                                        pt[:, 512 * hh + q0:512 * hh + q0 + 128],
                                        pt[:, 512 * hh + q0:512 * hh + q0 + 128],
                                        pattern=[[1, 128]],
                                        compare_op=AluOpType.is_ge, fill=0.0,
                                        base=512 * qg + q0 - 128 * kc,
                                        channel_multiplier=-1)
                            ptq[kc] = (pt, q0)
                    # evacuate ppv fast: BOTH attn-out+denom copies first
                    # (they gate PSUM reuse), then the cheap reciprocals
                    daos = []
                    for hh in range(2):
                        h = 2 * hp + hh
                        dao = nrmp.tile([128, 512], f32, tag="dao", name="dao")
                        nc.vector.tensor_copy(dao[:], ppv[h][:])
                        daos.append(dao)
                    recf = nrm2p.tile([1, 1024], f32, tag="rec", name="rec")
                    for hh in range(2):
                        nc.vector.reciprocal_approx_fast(
                            recf[0:1, 512 * hh:512 * (hh + 1)],
                            daos[hh][0:1, :])
                    recr = nrm2p.tile([1, 1024], f32r, tag="recr", name="recr")
                    nc.vector.tensor_copy(recr[:], recf[:])

                    def mk_norm(qg=qg, c2=hp, rec=recr, daos=daos):
                        def emit():
                            # two accumulating matmuls broadcast BOTH
                            # heads' 1/denom into one bc bank, then
                            # normalize into aot
                            bc = ps_bc.tile([128, 512], f32, tag="bc", name="bc")
                            for hh in range(2):
                                nc.tensor.matmul(
                                    bc[:],
                                    sel2[0:1, 128 * hh:128 * (hh + 1)],
                                    rec[0:1, 512 * hh:512 * (hh + 1)],
                                    start=(hh == 0), stop=(hh == 1))
                            for hh in range(2):
                                nc.vector.tensor_mul(
                                    aot[c2][64 * hh:64 * hh + 64,
                                            512 * qg:512 * (qg + 1)],
                                    daos[hh][64:128, :],
                                    bc[64 * hh:64 * hh + 64, :])
                        return emit
                    pending.append(mk_norm())
            # tail: the remaining normalize, then the last four
            # out-projection chunks out of wide ps_s tiles; split the
            # evacuation across ACT + DVE and the writeback DMAs across
            # the sync + gpsimd queues.
            while pending:
                emit_pending_one()
            for i in range(4):
                sc = 12 + i
                ps2 = ps_s.tile([128, 1024], f32, tag="ps", name="ps")
                for eg in range(2):
                    for c2 in range(2):
                        nc.tensor.matmul(
                            ps2[:, 512 * eg:512 * (eg + 1)],
                            aot[c2][:, 128 * sc:128 * (sc + 1)],
                            wov[:, c2, 512 * eg:512 * (eg + 1)],
                            start=(c2 == 0), stop=(c2 == 1))
                ysb = ysbp.tile([128, D], f16, tag="ysb", name="ysb")
                nc.scalar.copy(ysb[:, 0:512], ps2[:, 0:512])
                nc.vector.tensor_copy(ysb[:, 512:1024], ps2[:, 512:1024])
                for half in range(2):
                    sl = slice(512 * half, 512 * (half + 1))
                    eng = nc.sync if half == 0 else nc.gpsimd
                    eng.dma_start(
                        y[128 * sc:128 * (sc + 1), sl], ysb[:, sl])

    nc.compile()
    return nc


def _prep_inputs(x, token_positions, Wq, Wk, Wv, Wo):
    # even/odd interleave permutation within each head (for rotate-half RoPE)
    perm = np.concatenate([np.arange(0, DK, 2), np.arange(1, DK, 2)])

    pos = np.asarray(token_positions).astype(np.float32)
    angles = THETA ** (-np.arange(32, dtype=np.float32) / 32.0)
    ang = pos[:, None] * angles[None, :]          # [S, 32]
    cos32 = np.cos(ang).T.astype(np.float32)      # [32, S]
    sin32 = np.sin(ang).T.astype(np.float32)
    cos128 = np.concatenate([cos32, cos32, cos32, cos32], axis=0)
    sin128 = np.concatenate([-sin32, sin32, -sin32, sin32], axis=0)
    cos128 = np.ascontiguousarray(cos128).astype(np.float16)
    sin128 = np.ascontiguousarray(sin128).astype(np.float16)

    Wq = np.asarray(Wq, dtype=np.float32)
    Wk = np.asarray(Wk, dtype=np.float32)
    Wv = np.asarray(Wv, dtype=np.float32)
    Wo = np.asarray(Wo, dtype=np.float32)
    x = np.asarray(x, dtype=np.float32)

    f16 = np.float16

    def pack_w(wT):
        # [1024 d, 256 e] -> [128 p, 8 dc, 256 e]
        return np.ascontiguousarray(
            wT.reshape(8, 128, EL).transpose(1, 0, 2).reshape(128, 8 * EL)
        ).astype(f16)

    sel2 = np.zeros((1, 256), dtype=np.float32)
    sel2[0, 0:64] = 1.0
    sel2[0, 192:256] = 1.0

    in_maps = []
    for c in range(N_CORES):
        b = c // 4
        h0 = (c % 4) * HL
        esl = slice(h0 * DK, (h0 + HL) * DK)
        wq_h = Wq[esl].reshape(HL, DK, D)[:, perm].reshape(EL, D)
        wk_h = Wk[esl].reshape(HL, DK, D)[:, perm].reshape(EL, D)
        wv_h = Wv[esl]
        xT = x[b].T  # [1024 d, 2048 s]
        xt_p = np.ascontiguousarray(
            xT.reshape(8, 128, 4, 512).transpose(2, 1, 0, 3)).astype(f16)
        woT = Wo[:, esl].T  # [256 e, 1024 d_out]
        wo_p = np.ascontiguousarray(
            woT.reshape(2, 128, D).transpose(1, 0, 2).reshape(128, 2 * D)
        ).astype(f16)
        in_maps.append({
            "xt": xt_p,
            "wq": pack_w(wq_h.T),
            "wk": pack_w(wk_h.T),
            "wv": pack_w(wv_h.T),
            "wo": wo_p,
            "cosT": cos128,
            "sinT": sin128,
            "sel2": sel2,
        })
    return in_maps


def kernel(x, token_positions, Wq, Wk, Wv, Wo, _trace=False):
    from concourse.bass_utils import run_bass_kernel_spmd

    global _compiled
    if _compiled is None:
        _compiled = _build()
    in_maps = _prep_inputs(x, token_positions, Wq, Wk, Wv, Wo)
    res = run_bass_kernel_spmd(_compiled, in_maps, list(range(N_CORES)),
                               trace=_trace)
    parts = [res.results[c]["y"].astype(np.float64) for c in range(N_CORES)]
    out = np.empty((2, S, D), dtype=np.float32)
    out[0] = (parts[0] + parts[1] + parts[2] + parts[3]).astype(np.float32)
    out[1] = (parts[4] + parts[5] + parts[6] + parts[7]).astype(np.float32)
    if _trace:
        return out, res
    return out


# revision 25
# speedup vs baseline: 1.0441x; 1.0087x over previous
"""Trainium2 Bass kernel: multi-head self-attention with RoPE, causal mask.

Reference semantics (B=2, S=2048, D=1024, H=16, DK=64):
    q = rope(x @ Wq.T), k = rope(x @ Wk.T), v = x @ Wv.T   (per-head views)
    out = softmax(causal(q k^T / 8)) v ;  y = out @ Wo.T

Sharding over 8 cores: 2-way batch x 4-way heads (4 heads/core).
Each core computes a partial y [S, D] (its heads' contribution); host sums
the 4 partials per batch (device output is fp16, summed in fp64 on host).

On-device layout strategy (per core):
  - all 16-bit operands are fp16; host prepacks every weight/input so each
    SBUF tensor loads with ONE wide DMA (xT in 4 per-sg transfers); all
    loads ride the scalar queue in arrival-priority order (cos/sin last --
    the DVE-side rope needs them long after the PE needs x), sync carries
    only SBUF-SBUF swaps + y writebacks
  - stage A is emitted per-512-column group (sg): K ec0 + Q ec0 + four V
    chunks, so the PE streams as soon as each sg's xT slice lands; V is
    projected TRANSPOSED directly (lhsT = x chunk) -- no PE transpose
    pass; one strided copy scatters all 4 heads into the V working layout
    (ones col 0 for the softmax denominator, data at cols 64..127)
  - K lands naturally as [dk-pair, s] in ONE tile (h0 rows 0:64, h1 rows
    64:128); Q is written BLOCK-INTERLEAVED per 512-q-group with the other
    head's rows zeroed, so each non-diagonal score tile is a single
    1024-col matmul covering both heads (the zeros live on the Q side)
  - attention is HEAD-PAIR-OUTER with the ec1 K/Q projection chunks and
    the out-projection interleaved into the kc streams as PE filler;
    causal masks only touch the true 128-col diagonal sub-block
  - PSUM pools are shared across both stages (no release barrier between
    projection and attention)
  - normalization: reciprocal_approx_fast for both heads into one row,
    one f32r rounding copy, two accumulating PE matmuls broadcast into a
    single bc bank; multiply deferred into the next stream's slack
"""

import sys

sys.path.insert(0, "/opt/trn_rl_repo")

import numpy as np


S = 2048
D = 1024
NH = 16
DK = 64
HL = 4          # heads per core
EL = HL * DK    # 256 local e-dims
N_CORES = 8
THETA = 10000.0

_compiled = None


def _build():
    import concourse.bacc as bacc
    import concourse.tile as tile
    from concourse import mybir
    from concourse.alu_op_type import AluOpType

    dt = mybir.dt
    f32, f32r = dt.float32, dt.float32r
    f16 = dt.float16

    nc = bacc.Bacc("TRN2", target_bir_lowering=False, debug=False,
                   num_devices=N_CORES)

    xt_d = nc.dram_tensor("xt", [4, 128, 8, 512], f16, kind="ExternalInput").ap()
    wq_d = nc.dram_tensor("wq", [128, 8 * EL], f16, kind="ExternalInput").ap()
    wk_d = nc.dram_tensor("wk", [128, 8 * EL], f16, kind="ExternalInput").ap()
    wv_d = nc.dram_tensor("wv", [128, 8 * EL], f16, kind="ExternalInput").ap()
    wo_d = nc.dram_tensor("wo", [128, 2 * D], f16, kind="ExternalInput").ap()
    cos_d = nc.dram_tensor("cosT", [128, S], f16, kind="ExternalInput").ap()
    sin_d = nc.dram_tensor("sinT", [128, S], f16, kind="ExternalInput").ap()
    sel_d = nc.dram_tensor("sel2", [1, 256], f32r, kind="ExternalInput").ap()
    y = nc.dram_tensor("y", [S, D], f16, kind="ExternalOutput").ap()

    with tile.TileContext(nc) as tc:
        with tc.tile_pool(name="persist", bufs=1) as pp, \
             tc.tile_pool(name="kq", bufs=2) as kqp, \
             tc.tile_pool(name="rope", bufs=3) as ropp, \
             tc.tile_pool(name="ptp", bufs=8) as ptp, \
             tc.tile_pool(name="nrm", bufs=4) as nrmp, \
             tc.tile_pool(name="nrm2", bufs=2) as nrm2p, \
             tc.tile_pool(name="ysb", bufs=2) as ysbp, \
             tc.tile_pool(name="ps_s", bufs=2, space="PSUM") as ps_s, \
             tc.tile_pool(name="ps_pv", bufs=2, space="PSUM") as ps_pv, \
             tc.tile_pool(name="ps_y", bufs=1, space="PSUM") as ps_y, \
             tc.tile_pool(name="ps_bc", bufs=1, space="PSUM") as ps_bc:

            # persistent SBUF tiles (live across both stages)
            qt2 = [pp.tile([128, 2 * S], f16, tag=f"qt{c}", name=f"qt{c}") for c in range(2)]
            ktz = [pp.tile([128, S], f16, tag=f"ktz{c}", name=f"ktz{c}") for c in range(2)]
            vh_all = pp.tile([128, HL * 16 * 128], f16, tag="vh", name="vh")
            cos_sb = pp.tile([128, S], f16, tag="cos", name="cos")
            sin_sb = pp.tile([128, S], f16, tag="sin", name="sin")
            xt_all = pp.tile([128, 8 * S], f16, tag="xt", name="xt")
            wv_all = pp.tile([128, 8 * EL], f16, tag="wv", name="wv")
            wk_all = pp.tile([128, 8 * EL], f16, tag="wk", name="wk")
            wq_all = pp.tile([128, 8 * EL], f16, tag="wq", name="wq")
            wo_all = pp.tile([128, 2 * D], f16, tag="wo", name="wo")
            warm = pp.tile([128, 256], f16, tag="warm", name="warm")
            aot = [pp.tile([128, S], f16, tag=f"aot{c}", name=f"aot{c}") for c in range(2)]
            sel2 = pp.tile([1, 256], f32r, tag="sel2", name="sel2")

            xtv = xt_all[:].rearrange("p (d s) -> p d s", d=8)
            wvv = wv_all[:].rearrange("p (d e) -> p d e", d=8)
            wkv = wk_all[:].rearrange("p (d e) -> p d e", d=8)
            wqv = wq_all[:].rearrange("p (d e) -> p d e", d=8)
            wov = wo_all[:].rearrange("p (c d) -> p c d", c=2)
            vhv = vh_all[:].rearrange("p (h s c) -> p h s c", h=HL, c=128)
            qv = [qt2[c][:].rearrange("p (g h q) -> p g h q", h=2, q=512)
                  for c in range(2)]

            # ---- input DMA program ----
            nc.scalar.dma_start(wv_all[:], wv_d[:])
            nc.scalar.dma_start(xtv[:, :, 0:512], xt_d[0])
            nc.scalar.dma_start(wk_all[:], wk_d[:])
            nc.scalar.dma_start(xtv[:, :, 512:1024], xt_d[1])
            nc.scalar.dma_start(wq_all[:], wq_d[:])
            nc.scalar.dma_start(cos_sb[:], cos_d[:])
            nc.scalar.dma_start(sin_sb[:], sin_d[:])
            nc.scalar.dma_start(xtv[:, :, 1024:1536], xt_d[2])
            nc.scalar.dma_start(xtv[:, :, 1536:2048], xt_d[3])
            nc.scalar.dma_start(wo_all[:], wo_d[:])
            nc.scalar.dma_start(sel2[:], sel_d[:])

            # rope chunk: evacuate PSUM proj, SBUF-to-SBUF DMA block swap to
            # build the rotate-half partner, cos (DVE) / sin (gpsimd)
            # multiplies, adds into K or block-interleaved Q (DVE)
            def rope_chunk(ps, qa, qas, sg, is_k, ec, evac):
                sl = slice(512 * sg, 512 * (sg + 1))
                evac(qa[:, sl], ps[:])
                for blk in range(2):
                    b0 = 64 * blk
                    nc.sync.dma_start(
                        qas[b0:b0 + 32, sl], qa[b0 + 32:b0 + 64, sl])
                    nc.sync.dma_start(
                        qas[b0 + 32:b0 + 64, sl], qa[b0:b0 + 32, sl])
                qc = ropp.tile([128, 512], f16, tag="qc", name="qc")
                qs = ropp.tile([128, 512], f16, tag="qs", name="qs")
                nc.vector.tensor_mul(qc[:], qa[:, sl], cos_sb[:, sl])
                nc.gpsimd.tensor_mul(qs[:], qas[:, sl], sin_sb[:, sl])
                if is_k:
                    nc.vector.tensor_add(
                        ktz[ec][0:64, sl], qc[0:64, :], qs[0:64, :])
                    nc.vector.tensor_add(
                        ktz[ec][64:128, sl], qc[64:128, :], qs[64:128, :])
                else:
                    nc.vector.tensor_add(
                        qv[ec][0:64, sg, 0, :], qc[0:64, :], qs[0:64, :])
                    nc.vector.tensor_add(
                        qv[ec][64:128, sg, 1, :], qc[64:128, :], qs[64:128, :])

            # ======== stage A: per-sg V + K/Q ec0 projections ========
            # warm up the PE clock-gate while input DMAs land
            nc.vector.memset(warm[:], 0.0)
            wp = ps_y.tile([128, 512], f32, tag="yp", name="yp")
            for _ in range(26):
                nc.tensor.matmul(wp[:, 0:256], warm[:, 0:128], warm[:],
                                 start=True, stop=True)

            # zero the other-head rows of the block-interleaved Q + the
            # softmax-denominator ones column (gpsimd is idle here)
            for c in range(2):
                nc.gpsimd.memset(qv[c][64:128, :, 0, :], 0.0)
                nc.gpsimd.memset(qv[c][0:64, :, 1, :], 0.0)
            nc.gpsimd.memset(vhv[:, :, :, 0:1], 1.0)

            kq_qa = {}
            for is_k in (True, False):
                kq_qa[is_k] = (
                    kqp.tile([128, S], f16, tag="qa", name="qa"),
                    kqp.tile([128, S], f16, tag="qas", name="qas"))

            def kq_chunk(sg, is_k, wsv):
                sl = slice(512 * sg, 512 * (sg + 1))
                ps = ps_s.tile([128, 1024], f32, tag="ps", name="ps")
                for dc in range(8):
                    nc.tensor.matmul(
                        ps[:, 0:512], wsv[:, dc, 0:128], xtv[:, dc, sl],
                        start=(dc == 0), stop=(dc == 7))
                qa, qas = kq_qa[is_k]
                rope_chunk(ps[:, 0:512], qa, qas, sg, is_k, 0,
                           evac=nc.scalar.copy)

            def v_chunk(sc):
                psv = ps_pv.tile([128, 512], f32, tag="ppv", name="ppv")
                for dc in range(8):
                    nc.tensor.matmul(
                        psv[:, 0:256],
                        xtv[:, dc, 128 * sc:128 * (sc + 1)],
                        wvv[:, dc, :],
                        start=(dc == 0), stop=(dc == 7))
                # always ACT: the DVE queue holds rope multiplies that wait
                # on cos/sin -- a DVE evac behind them would HOL-block the
                # PE on PSUM reuse
                nc.scalar.copy(vhv[:, :, sc, 64:128],
                               psv[:, 0:256].rearrange("p (h e) -> p h e", h=HL))

            for sg in range(4):
                if sg == 0:
                    # sg0: V first -- wv+xt0 land before wk
                    for i in range(4):
                        v_chunk(i)
                    kq_chunk(0, True, wkv)
                    kq_chunk(0, False, wqv)
                else:
                    kq_chunk(sg, True, wkv)
                    kq_chunk(sg, False, wqv)
                    for i in range(4):
                        v_chunk(4 * sg + i)

            # ======== stage B: attention (head-pair outer) + out-proj ========
            # ec1 projection chunks, emitted as PE filler inside head-
            # pair 0's attention stream (use the out-projection's PSUM
            # slot, which is idle until head-pair 1)
            kq_tiles = {}
            for is_k in (True, False):
                kq_tiles[is_k] = (
                    kqp.tile([128, S], f16, tag="qa", name="qa"),
                    kqp.tile([128, S], f16, tag="qas", name="qas"))

            def mk_proj_pieces(is_k, sg):
                box = {}

                def piece1():
                    wsv = wkv if is_k else wqv
                    box["ps"] = ps_y.tile([128, 512], f32, tag="yp", name="yp")
                    for dc in range(4):
                        nc.tensor.matmul(
                            box["ps"][:],
                            wsv[:, dc, 128:256],
                            xtv[:, dc, 512 * sg:512 * (sg + 1)],
                            start=(dc == 0), stop=False)

                def piece2():
                    wsv = wkv if is_k else wqv
                    qa, qas = kq_tiles[is_k]
                    for dc in range(4, 8):
                        nc.tensor.matmul(
                            box["ps"][:],
                            wsv[:, dc, 128:256],
                            xtv[:, dc, 512 * sg:512 * (sg + 1)],
                            start=False, stop=(dc == 7))
                    rope_chunk(ps=box["ps"], qa=qa, qas=qas, sg=sg,
                               is_k=is_k, ec=1, evac=nc.vector.tensor_copy)
                return [piece1, piece2]

            filler_q = []
            for is_k in (True, False):
                for sg in range(4):
                    filler_q += mk_proj_pieces(is_k, sg)

            def out_proj_eg(sc, eg, ysb):
                yp = ps_y.tile([128, 512], f32, tag="yp", name="yp")
                for c2 in range(2):
                    nc.tensor.matmul(
                        yp[:],
                        aot[c2][:, 128 * sc:128 * (sc + 1)],
                        wov[:, c2, 512 * eg:512 * (eg + 1)],
                        start=(c2 == 0), stop=(c2 == 1))
                nc.vector.tensor_copy(
                    ysb[:, 512 * eg:512 * (eg + 1)], yp[:])
                if eg == 1:
                    for half in range(2):
                        sl = slice(512 * half, 512 * (half + 1))
                        nc.sync.dma_start(
                            y[128 * sc:128 * (sc + 1), sl], ysb[:, sl])

            def out_proj_pieces(sc):
                box = {}

                def p1():
                    box["ysb"] = ysbp.tile([128, D], f16, tag="ysb", name="ysb")
                    out_proj_eg(sc, 0, box["ysb"])

                def p2():
                    out_proj_eg(sc, 1, box["ysb"])
                return [p1, p2]

            pending = []   # deferred normalize closures

            def emit_pending_one():
                if pending:
                    pending.pop(0)()

            SKEW = 3
            for hp in range(2):
                for qg in range(4):
                    n_kc = 4 * qg + 4
                    # flush the previous stream's normalize early, then
                    # enqueue that q-group's out-projection pieces as
                    # per-kc PE filler
                    norm_at = {} if (hp, qg) == (0, 0) else {1: 1}
                    if hp == 1 and qg >= 1:
                        for sc in range(4 * (qg - 1), 4 * qg):
                            filler_q += out_proj_pieces(sc)
                    ppv = {}
                    for hh in range(2):
                        h = 2 * hp + hh
                        ppv[h] = ps_pv.tile([128, 512], f32, tag="ppv", name="ppv")
                    ptq = {}
                    for kc in range(n_kc + SKEW):
                        for _ in range(norm_at.get(kc, 0)):
                            emit_pending_one()
                        if kc >= 2 and filler_q:
                            filler_q.pop(0)()
                        # PV first: keeps queued work ahead of a score
                        # matmul that may block on PSUM reuse
                        kcp = kc - SKEW
                        if kcp >= 0:
                            ptv2, q0v = ptq.pop(kcp)
                            for hh in range(2):
                                h = 2 * hp + hh
                                nc.tensor.matmul(
                                    ppv[h][:, q0v:512],
                                    vhv[:, h, kcp, :],
                                    ptv2[:, 512 * hh + q0v:512 * (hh + 1)],
                                    start=(kcp == 0), stop=(kcp == n_kc - 1))
                        if kc < n_kc:
                            # diagonal tiles only need q >= k
                            r = kc - 4 * qg
                            q0 = 128 * r if r > 0 else 0
                            ps2 = ps_s.tile([128, 1024], f32, tag="ps", name="ps")
                            # per-head matmuls; the other head's rows are
                            # zero on the Q side (max matmul N is one bank)
                            for hh in range(2):
                                nc.tensor.matmul(
                                    ps2[:, 512 * hh + q0:512 * (hh + 1)],
                                    ktz[hp][:, 128 * kc:128 * (kc + 1)],
                                    qv[hp][:, qg, hh, q0:512],
                                    start=True, stop=True)
                            pt = ptp.tile([128, 1024], f16, tag="pt", name="pt")
                            psv2 = ps2[:].rearrange("p (h q) -> p h q", h=2)[:, :, q0:512]
                            ptv = pt[:].rearrange("p (h q) -> p h q", h=2)[:, :, q0:512]
                            nc.scalar.activation(
                                ptv, psv2,
                                mybir.ActivationFunctionType.Exp,
                                scale=0.125)
                            if r >= 0:
                                # only the 128-col diagonal sub-block can
                                # have q < k; the rest is already causal
                                for hh in range(2):
                                    nc.gpsimd.affine_select(
                                        pt[:, 512 * hh + q0:512 * hh + q0 + 128],
                                        pt[:, 512 * hh + q0:512 * hh + q0 + 128],
                                        pattern=[[1, 128]],
                                        compare_op=AluOpType.is_ge, fill=0.0,
                                        base=512 * qg + q0 - 128 * kc,
                                        channel_multiplier=-1)
                            ptq[kc] = (pt, q0)
                    if (hp, qg) == (1, 3):
                        # keep the PE p-state up through the serial norm
                        # window so the tail out-proj runs at full clock
                        wpd = ps_y.tile([128, 512], f32, tag="yp", name="yp")
                        for _ in range(12):
                            nc.tensor.matmul(wpd[:, 0:256], warm[:, 0:128],
                                             warm[:], start=True, stop=True)
                    # evacuate ppv fast: BOTH attn-out+denom copies first
                    # (they gate PSUM reuse), then the cheap reciprocals
                    last_qg = (hp, qg) == (1, 3)

                    def emit_recs():
                        # reciprocals read the denominator row straight
                        # from PSUM -- no wait on the dao evacuations
                        recf = nrm2p.tile([1, 1024], f32, tag="rec", name="rec")
                        for hh in range(2):
                            nc.vector.reciprocal_approx_fast(
                                recf[0:1, 512 * hh:512 * (hh + 1)],
                                ppv[2 * hp + hh][0:1, :])
                        recr = nrm2p.tile([1, 1024], f32r, tag="recr", name="recr")
                        nc.vector.tensor_copy(recr[:], recf[:])
                        return recr

                    def emit_daos():
                        daos = []
                        for hh in range(2):
                            h = 2 * hp + hh
                            dao = nrmp.tile([128, 512], f32, tag="dao", name="dao")
                            # in the tail ACT is idle: split the copies so
                            # the final norm chain isn't DVE-serialized
                            if last_qg and hh == 0:
                                nc.scalar.copy(dao[:], ppv[h][:])
                            else:
                                nc.vector.tensor_copy(dao[:], ppv[h][:])
                            daos.append(dao)
                        return daos

                    if last_qg:
                        recr = emit_recs()
                        daos = emit_daos()
                    else:
                        # dao copies first: they gate ppv PSUM reuse for
                        # the next stream
                        daos = emit_daos()
                        recr = emit_recs()

                    def mk_norm(qg=qg, c2=hp, rec=recr, daos=daos):
                        def emit():
                            # two accumulating matmuls broadcast BOTH
                            # heads' 1/denom into one bc bank, then
                            # normalize into aot
                            bc = ps_bc.tile([128, 512], f32, tag="bc", name="bc")
                            for hh in range(2):
                                nc.tensor.matmul(
                                    bc[:],
                                    sel2[0:1, 128 * hh:128 * (hh + 1)],
                                    rec[0:1, 512 * hh:512 * (hh + 1)],
                                    start=(hh == 0), stop=(hh == 1))
                            for hh in range(2):
                                nc.vector.tensor_mul(
                                    aot[c2][64 * hh:64 * hh + 64,
                                            512 * qg:512 * (qg + 1)],
                                    daos[hh][64:128, :],
                                    bc[64 * hh:64 * hh + 64, :])
                        return emit
                    pending.append(mk_norm())
            # tail: the remaining normalize, then the last four
            # out-projection chunks out of wide ps_s tiles; split the
            # evacuation across ACT + DVE and the writeback DMAs across
            # the sync + gpsimd queues.
            while pending:
                emit_pending_one()
            for i in range(4):
                sc = 12 + i
                ps2 = ps_s.tile([128, 1024], f32, tag="ps", name="ps")
                for eg in range(2):
                    for c2 in range(2):
                        nc.tensor.matmul(
                            ps2[:, 512 * eg:512 * (eg + 1)],
                            aot[c2][:, 128 * sc:128 * (sc + 1)],
                            wov[:, c2, 512 * eg:512 * (eg + 1)],
                            start=(c2 == 0), stop=(c2 == 1))
                ysb = ysbp.tile([128, D], f16, tag="ysb", name="ysb")
                nc.scalar.copy(ysb[:, 0:512], ps2[:, 0:512])
                nc.vector.tensor_copy(ysb[:, 512:1024], ps2[:, 512:1024])
                for half in range(2):
                    sl = slice(512 * half, 512 * (half + 1))
                    eng = nc.sync if half == 0 else nc.gpsimd
                    eng.dma_start(
                        y[128 * sc:128 * (sc + 1), sl], ysb[:, sl])

    nc.compile()
    return nc


def _prep_inputs(x, token_positions, Wq, Wk, Wv, Wo):
    # even/odd interleave permutation within each head (for rotate-half RoPE)
    perm = np.concatenate([np.arange(0, DK, 2), np.arange(1, DK, 2)])

    pos = np.asarray(token_positions).astype(np.float32)
    angles = THETA ** (-np.arange(32, dtype=np.float32) / 32.0)
    ang = pos[:, None] * angles[None, :]          # [S, 32]
    cos32 = np.cos(ang).T.astype(np.float32)      # [32, S]
    sin32 = np.sin(ang).T.astype(np.float32)
    cos128 = np.concatenate([cos32, cos32, cos32, cos32], axis=0)
    sin128 = np.concatenate([-sin32, sin32, -sin32, sin32], axis=0)
    cos128 = np.ascontiguousarray(cos128).astype(np.float16)
    sin128 = np.ascontiguousarray(sin128).astype(np.float16)

    Wq = np.asarray(Wq, dtype=np.float32)
    Wk = np.asarray(Wk, dtype=np.float32)
    Wv = np.asarray(Wv, dtype=np.float32)
    Wo = np.asarray(Wo, dtype=np.float32)
    x = np.asarray(x, dtype=np.float32)

    f16 = np.float16

    def pack_w(wT):
        # [1024 d, 256 e] -> [128 p, 8 dc, 256 e]
        return np.ascontiguousarray(
            wT.reshape(8, 128, EL).transpose(1, 0, 2).reshape(128, 8 * EL)
        ).astype(f16)

    sel2 = np.zeros((1, 256), dtype=np.float32)
    sel2[0, 0:64] = 1.0
    sel2[0, 192:256] = 1.0

    in_maps = []
    for c in range(N_CORES):
        b = c // 4
        h0 = (c % 4) * HL
        esl = slice(h0 * DK, (h0 + HL) * DK)
        wq_h = Wq[esl].reshape(HL, DK, D)[:, perm].reshape(EL, D)
        wk_h = Wk[esl].reshape(HL, DK, D)[:, perm].reshape(EL, D)
        wv_h = Wv[esl]
        xT = x[b].T  # [1024 d, 2048 s]
        xt_p = np.ascontiguousarray(
            xT.reshape(8, 128, 4, 512).transpose(2, 1, 0, 3)).astype(f16)
        woT = Wo[:, esl].T  # [256 e, 1024 d_out]
        wo_p = np.ascontiguousarray(
            woT.reshape(2, 128, D).transpose(1, 0, 2).reshape(128, 2 * D)
        ).astype(f16)
        in_maps.append({
            "xt": xt_p,
            "wq": pack_w(wq_h.T),
            "wk": pack_w(wk_h.T),
            "wv": pack_w(wv_h.T),
            "wo": wo_p,
            "cosT": cos128,
            "sinT": sin128,
            "sel2": sel2,
        })
    return in_maps


def kernel(x, token_positions, Wq, Wk, Wv, Wo, _trace=False):
    from concourse.bass_utils import run_bass_kernel_spmd

    global _compiled
    if _compiled is None:
        _compiled = _build()
    in_maps = _prep_inputs(x, token_positions, Wq, Wk, Wv, Wo)
    res = run_bass_kernel_spmd(_compiled, in_maps, list(range(N_CORES)),
                               trace=_trace)
    parts = [res.results[c]["y"].astype(np.float64) for c in range(N_CORES)]
    out = np.empty((2, S, D), dtype=np.float32)
    out[0] = (parts[0] + parts[1] + parts[2] + parts[3]).astype(np.float32)
    out[1] = (parts[4] + parts[5] + parts[6] + parts[7]).astype(np.float32)
    if _trace:
        return out, res
    return out


# revision 28
# speedup vs baseline: 1.0500x; 1.0057x over previous
"""Trainium2 Bass kernel: multi-head self-attention with RoPE, causal mask.

Reference semantics (B=2, S=2048, D=1024, H=16, DK=64):
    q = rope(x @ Wq.T), k = rope(x @ Wk.T), v = x @ Wv.T   (per-head views)
    out = softmax(causal(q k^T / 8)) v ;  y = out @ Wo.T

Sharding over 8 cores: 2-way batch x 4-way heads (4 heads/core).
Each core computes a partial y [S, D] (its heads' contribution); host sums
the 4 partials per batch (device output is fp16, summed in fp64 on host).

On-device layout strategy (per core):
  - all 16-bit operands are fp16; host prepacks every weight/input so each
    SBUF tensor loads with ONE wide DMA (xT in 4 per-sg transfers); all
    loads ride the scalar queue in arrival-priority order (cos/sin last --
    the DVE-side rope needs them long after the PE needs x), sync carries
    only SBUF-SBUF swaps + y writebacks
  - stage A is emitted per-512-column group (sg): K ec0 + Q ec0 + four V
    chunks, so the PE streams as soon as each sg's xT slice lands; V is
    projected TRANSPOSED directly (lhsT = x chunk) -- no PE transpose
    pass; one strided copy scatters all 4 heads into the V working layout
    (ones col 0 for the softmax denominator, data at cols 64..127)
  - K lands naturally as [dk-pair, s] in ONE tile (h0 rows 0:64, h1 rows
    64:128); Q is written BLOCK-INTERLEAVED per 512-q-group with the other
    head's rows zeroed, so each non-diagonal score tile is a single
    1024-col matmul covering both heads (the zeros live on the Q side)
  - attention is HEAD-PAIR-OUTER with the ec1 K/Q projection chunks and
    the out-projection interleaved into the kc streams as PE filler;
    causal masks only touch the true 128-col diagonal sub-block
  - PSUM pools are shared across both stages (no release barrier between
    projection and attention)
  - normalization: reciprocal_approx_fast for both heads into one row,
    one f32r rounding copy, two accumulating PE matmuls broadcast into a
    single bc bank; multiply deferred into the next stream's slack
"""

import sys

sys.path.insert(0, "/opt/trn_rl_repo")

import numpy as np


S = 2048
D = 1024
NH = 16
DK = 64
HL = 4          # heads per core
EL = HL * DK    # 256 local e-dims
N_CORES = 8
THETA = 10000.0

_compiled = None


def _build():
    import concourse.bacc as bacc
    import concourse.tile as tile
    from concourse import mybir
    from concourse.alu_op_type import AluOpType

    dt = mybir.dt
    f32, f32r = dt.float32, dt.float32r
    f16 = dt.float16

    nc = bacc.Bacc("TRN2", target_bir_lowering=False, debug=False,
                   num_devices=N_CORES)

    xt_d = nc.dram_tensor("xt", [4, 128, 8, 512], f16, kind="ExternalInput").ap()
    wq_d = nc.dram_tensor("wq", [128, 8 * EL], f16, kind="ExternalInput").ap()
    wk_d = nc.dram_tensor("wk", [128, 8 * EL], f16, kind="ExternalInput").ap()
    wv_d = nc.dram_tensor("wv", [128, 8 * EL], f16, kind="ExternalInput").ap()
    wo_d = nc.dram_tensor("wo", [128, 2 * D], f16, kind="ExternalInput").ap()
    cos_d = nc.dram_tensor("cosT", [128, S], f16, kind="ExternalInput").ap()
    sin_d = nc.dram_tensor("sinT", [128, S], f16, kind="ExternalInput").ap()
    sel_d = nc.dram_tensor("sel2", [1, 256], f32r, kind="ExternalInput").ap()
    y = nc.dram_tensor("y", [S, D], f16, kind="ExternalOutput").ap()

    with tile.TileContext(nc) as tc:
        with tc.tile_pool(name="persist", bufs=1) as pp, \
             tc.tile_pool(name="kq", bufs=2) as kqp, \
             tc.tile_pool(name="rope", bufs=3) as ropp, \
             tc.tile_pool(name="ptp", bufs=8) as ptp, \
             tc.tile_pool(name="nrm", bufs=4) as nrmp, \
             tc.tile_pool(name="nrm2", bufs=2) as nrm2p, \
             tc.tile_pool(name="ysb", bufs=2) as ysbp, \
             tc.tile_pool(name="ps_s", bufs=2, space="PSUM") as ps_s, \
             tc.tile_pool(name="ps_pv", bufs=2, space="PSUM") as ps_pv, \
             tc.tile_pool(name="ps_y", bufs=1, space="PSUM") as ps_y, \
             tc.tile_pool(name="ps_bc", bufs=1, space="PSUM") as ps_bc:

            # persistent SBUF tiles (live across both stages)
            qt2 = [pp.tile([128, 2 * S], f16, tag=f"qt{c}", name=f"qt{c}") for c in range(2)]
            ktz = [pp.tile([128, S], f16, tag=f"ktz{c}", name=f"ktz{c}") for c in range(2)]
            vh_all = pp.tile([128, HL * 16 * 128], f16, tag="vh", name="vh")
            cos_sb = pp.tile([128, S], f16, tag="cos", name="cos")
            sin_sb = pp.tile([128, S], f16, tag="sin", name="sin")
            xt_all = pp.tile([128, 8 * S], f16, tag="xt", name="xt")
            wv_all = pp.tile([128, 8 * EL], f16, tag="wv", name="wv")
            wk_all = pp.tile([128, 8 * EL], f16, tag="wk", name="wk")
            wq_all = pp.tile([128, 8 * EL], f16, tag="wq", name="wq")
            wo_all = pp.tile([128, 2 * D], f16, tag="wo", name="wo")
            warm = pp.tile([128, 256], f16, tag="warm", name="warm")
            aot = [pp.tile([128, S], f16, tag=f"aot{c}", name=f"aot{c}") for c in range(2)]
            sel2 = pp.tile([1, 256], f32r, tag="sel2", name="sel2")

            xtv = xt_all[:].rearrange("p (d s) -> p d s", d=8)
            wvv = wv_all[:].rearrange("p (d e) -> p d e", d=8)
            wkv = wk_all[:].rearrange("p (d e) -> p d e", d=8)
            wqv = wq_all[:].rearrange("p (d e) -> p d e", d=8)
            wov = wo_all[:].rearrange("p (c d) -> p c d", c=2)
            vhv = vh_all[:].rearrange("p (h s c) -> p h s c", h=HL, c=128)
            qv = [qt2[c][:].rearrange("p (g h q) -> p g h q", h=2, q=512)
                  for c in range(2)]

            # ---- input DMA program ----
            nc.scalar.dma_start(wv_all[:], wv_d[:])
            nc.scalar.dma_start(xtv[:, :, 0:512], xt_d[0])
            nc.scalar.dma_start(wk_all[:], wk_d[:])
            nc.scalar.dma_start(xtv[:, :, 512:1024], xt_d[1])
            nc.scalar.dma_start(wq_all[:], wq_d[:])
            nc.scalar.dma_start(cos_sb[:], cos_d[:])
            nc.scalar.dma_start(sin_sb[:], sin_d[:])
            nc.scalar.dma_start(xtv[:, :, 1024:1536], xt_d[2])
            nc.scalar.dma_start(xtv[:, :, 1536:2048], xt_d[3])
            nc.scalar.dma_start(wo_all[:], wo_d[:])
            nc.scalar.dma_start(sel2[:], sel_d[:])

            # rope chunk: evacuate PSUM proj, SBUF-to-SBUF DMA block swap to
            # build the rotate-half partner, cos (DVE) / sin (gpsimd)
            # multiplies, adds into K or block-interleaved Q (DVE)
            def rope_chunk(ps, qa, qas, sg, is_k, ec, evac):
                sl = slice(512 * sg, 512 * (sg + 1))
                evac(qa[:, sl], ps[:])
                for blk in range(2):
                    b0 = 64 * blk
                    nc.sync.dma_start(
                        qas[b0:b0 + 32, sl], qa[b0 + 32:b0 + 64, sl])
                    nc.sync.dma_start(
                        qas[b0 + 32:b0 + 64, sl], qa[b0:b0 + 32, sl])
                qc = ropp.tile([128, 512], f16, tag="qc", name="qc")
                qs = ropp.tile([128, 512], f16, tag="qs", name="qs")
                nc.vector.tensor_mul(qc[:], qa[:, sl], cos_sb[:, sl])
                nc.gpsimd.tensor_mul(qs[:], qas[:, sl], sin_sb[:, sl])
                if is_k:
                    nc.vector.tensor_add(
                        ktz[ec][0:64, sl], qc[0:64, :], qs[0:64, :])
                    nc.vector.tensor_add(
                        ktz[ec][64:128, sl], qc[64:128, :], qs[64:128, :])
                else:
                    nc.vector.tensor_add(
                        qv[ec][0:64, sg, 0, :], qc[0:64, :], qs[0:64, :])
                    nc.vector.tensor_add(
                        qv[ec][64:128, sg, 1, :], qc[64:128, :], qs[64:128, :])

            # ======== stage A: per-sg V + K/Q ec0 projections ========
            # warm up the PE clock-gate while input DMAs land
            nc.vector.memset(warm[:], 0.0)
            wp = ps_y.tile([128, 512], f32, tag="yp", name="yp")
            for _ in range(34):
                nc.tensor.matmul(wp[:, 0:256], warm[:, 0:128], warm[:],
                                 start=True, stop=True)

            # zero the other-head rows of the block-interleaved Q + the
            # softmax-denominator ones column (gpsimd is idle here)
            for c in range(2):
                nc.gpsimd.memset(qv[c][64:128, :, 0, :], 0.0)
                nc.gpsimd.memset(qv[c][0:64, :, 1, :], 0.0)
            nc.gpsimd.memset(vhv[:, :, :, 0:1], 1.0)

            kq_qa = {}
            for is_k in (True, False):
                kq_qa[is_k] = (
                    kqp.tile([128, S], f16, tag="qa", name="qa"),
                    kqp.tile([128, S], f16, tag="qas", name="qas"))

            def kq_chunk(sg, is_k, wsv):
                sl = slice(512 * sg, 512 * (sg + 1))
                ps = ps_s.tile([128, 1024], f32, tag="ps", name="ps")
                for dc in range(8):
                    nc.tensor.matmul(
                        ps[:, 0:512], wsv[:, dc, 0:128], xtv[:, dc, sl],
                        start=(dc == 0), stop=(dc == 7))
                qa, qas = kq_qa[is_k]
                rope_chunk(ps[:, 0:512], qa, qas, sg, is_k, 0,
                           evac=nc.scalar.copy)

            def v_chunk(sc):
                psv = ps_pv.tile([128, 512], f32, tag="ppv", name="ppv")
                for dc in range(8):
                    nc.tensor.matmul(
                        psv[:, 0:256],
                        xtv[:, dc, 128 * sc:128 * (sc + 1)],
                        wvv[:, dc, :],
                        start=(dc == 0), stop=(dc == 7))
                # always ACT: the DVE queue holds rope multiplies that wait
                # on cos/sin -- a DVE evac behind them would HOL-block the
                # PE on PSUM reuse
                nc.scalar.copy(vhv[:, :, sc, 64:128],
                               psv[:, 0:256].rearrange("p (h e) -> p h e", h=HL))

            for sg in range(4):
                if sg == 0:
                    # sg0: V first -- wv+xt0 land before wk
                    for i in range(4):
                        v_chunk(i)
                    kq_chunk(0, True, wkv)
                    kq_chunk(0, False, wqv)
                else:
                    kq_chunk(sg, True, wkv)
                    kq_chunk(sg, False, wqv)
                    for i in range(4):
                        v_chunk(4 * sg + i)

            # ======== stage B: attention (head-pair outer) + out-proj ========
            # ec1 projection chunks, emitted as PE filler inside head-
            # pair 0's attention stream (use the out-projection's PSUM
            # slot, which is idle until head-pair 1)
            kq_tiles = {}
            for is_k in (True, False):
                kq_tiles[is_k] = (
                    kqp.tile([128, S], f16, tag="qa", name="qa"),
                    kqp.tile([128, S], f16, tag="qas", name="qas"))

            def mk_proj_pieces(is_k, sg):
                box = {}

                def piece1():
                    wsv = wkv if is_k else wqv
                    box["ps"] = ps_y.tile([128, 512], f32, tag="yp", name="yp")
                    for dc in range(4):
                        nc.tensor.matmul(
                            box["ps"][:],
                            wsv[:, dc, 128:256],
                            xtv[:, dc, 512 * sg:512 * (sg + 1)],
                            start=(dc == 0), stop=False)

                def piece2():
                    wsv = wkv if is_k else wqv
                    qa, qas = kq_tiles[is_k]
                    for dc in range(4, 8):
                        nc.tensor.matmul(
                            box["ps"][:],
                            wsv[:, dc, 128:256],
                            xtv[:, dc, 512 * sg:512 * (sg + 1)],
                            start=False, stop=(dc == 7))
                    rope_chunk(ps=box["ps"], qa=qa, qas=qas, sg=sg,
                               is_k=is_k, ec=1, evac=nc.vector.tensor_copy)
                return [piece1, piece2]

            # hold back the Q ec1 sg2/sg3 pieces: head-pair 1's first
            # q-group has empty filler slots (its out-projection isn't
            # available yet), and these pieces aren't needed until the
            # hp1 qg2/qg3 score streams
            filler_q = []
            deferred_q = []
            for is_k in (True, False):
                for sg in range(4):
                    pieces = mk_proj_pieces(is_k, sg)
                    if not is_k and sg >= 2:
                        deferred_q += pieces
                    else:
                        filler_q += pieces

            def out_proj_eg(sc, eg, ysb):
                yp = ps_y.tile([128, 512], f32, tag="yp", name="yp")
                for c2 in range(2):
                    nc.tensor.matmul(
                        yp[:],
                        aot[c2][:, 128 * sc:128 * (sc + 1)],
                        wov[:, c2, 512 * eg:512 * (eg + 1)],
                        start=(c2 == 0), stop=(c2 == 1))
                nc.vector.tensor_copy(
                    ysb[:, 512 * eg:512 * (eg + 1)], yp[:])
                if eg == 1:
                    for half in range(2):
                        sl = slice(512 * half, 512 * (half + 1))
                        nc.sync.dma_start(
                            y[128 * sc:128 * (sc + 1), sl], ysb[:, sl])

            def out_proj_pieces(sc):
                box = {}

                def p1():
                    box["ysb"] = ysbp.tile([128, D], f16, tag="ysb", name="ysb")
                    out_proj_eg(sc, 0, box["ysb"])

                def p2():
                    out_proj_eg(sc, 1, box["ysb"])
                return [p1, p2]

            pending = []   # deferred normalize closures

            def emit_pending_one():
                if pending:
                    pending.pop(0)()

            SKEW = 3
            for hp in range(2):
                for qg in range(4):
                    n_kc = 4 * qg + 4
                    # flush the previous stream's normalize early, then
                    # enqueue that q-group's out-projection pieces as
                    # per-kc PE filler
                    norm_at = {} if (hp, qg) == (0, 0) else {1: 1}
                    if hp == 1 and qg == 0:
                        filler_q += deferred_q
                        deferred_q = []
                    if hp == 1 and qg >= 1:
                        for sc in range(4 * (qg - 1), 4 * qg):
                            filler_q += out_proj_pieces(sc)
                    ppv = {}
                    for hh in range(2):
                        h = 2 * hp + hh
                        ppv[h] = ps_pv.tile([128, 512], f32, tag="ppv", name="ppv")
                    ptq = {}
                    for kc in range(n_kc + SKEW):
                        for _ in range(norm_at.get(kc, 0)):
                            emit_pending_one()
                        if kc >= 2 and filler_q:
                            filler_q.pop(0)()
                        # PV first: keeps queued work ahead of a score
                        # matmul that may block on PSUM reuse
                        kcp = kc - SKEW
                        if kcp >= 0:
                            ptv2, q0v = ptq.pop(kcp)
                            for hh in range(2):
                                h = 2 * hp + hh
                                nc.tensor.matmul(
                                    ppv[h][:, q0v:512],
                                    vhv[:, h, kcp, :],
                                    ptv2[:, 512 * hh + q0v:512 * (hh + 1)],
                                    start=(kcp == 0), stop=(kcp == n_kc - 1))
                        if kc < n_kc:
                            # diagonal tiles only need q >= k
                            r = kc - 4 * qg
                            q0 = 128 * r if r > 0 else 0
                            ps2 = ps_s.tile([128, 1024], f32, tag="ps", name="ps")
                            # per-head matmuls; the other head's rows are
                            # zero on the Q side (max matmul N is one bank)
                            for hh in range(2):
                                nc.tensor.matmul(
                                    ps2[:, 512 * hh + q0:512 * (hh + 1)],
                                    ktz[hp][:, 128 * kc:128 * (kc + 1)],
                                    qv[hp][:, qg, hh, q0:512],
                                    start=True, stop=True)
                            pt = ptp.tile([128, 1024], f16, tag="pt", name="pt")
                            psv2 = ps2[:].rearrange("p (h q) -> p h q", h=2)[:, :, q0:512]
                            ptv = pt[:].rearrange("p (h q) -> p h q", h=2)[:, :, q0:512]
                            nc.scalar.activation(
                                ptv, psv2,
                                mybir.ActivationFunctionType.Exp,
                                scale=0.125)
                            if r >= 0:
                                # only the 128-col diagonal sub-block can
                                # have q < k; the rest is already causal
                                for hh in range(2):
                                    nc.gpsimd.affine_select(
                                        pt[:, 512 * hh + q0:512 * hh + q0 + 128],
                                        pt[:, 512 * hh + q0:512 * hh + q0 + 128],
                                        pattern=[[1, 128]],
                                        compare_op=AluOpType.is_ge, fill=0.0,
                                        base=512 * qg + q0 - 128 * kc,
                                        channel_multiplier=-1)
                            ptq[kc] = (pt, q0)
                    if (hp, qg) == (1, 3):
                        # keep the PE p-state up through the serial norm
                        # window so the tail out-proj runs at full clock
                        wpd = ps_y.tile([128, 512], f32, tag="yp", name="yp")
                        for _ in range(12):
                            nc.tensor.matmul(wpd[:, 0:256], warm[:, 0:128],
                                             warm[:], start=True, stop=True)
                    # evacuate ppv fast: BOTH attn-out+denom copies first
                    # (they gate PSUM reuse), then the cheap reciprocals
                    last_qg = (hp, qg) == (1, 3)

                    def emit_recs():
                        # reciprocals read the denominator row straight
                        # from PSUM -- no wait on the dao evacuations
                        recf = nrm2p.tile([1, 1024], f32, tag="rec", name="rec")
                        for hh in range(2):
                            nc.vector.reciprocal_approx_fast(
                                recf[0:1, 512 * hh:512 * (hh + 1)],
                                ppv[2 * hp + hh][0:1, :])
                        recr = nrm2p.tile([1, 1024], f32r, tag="recr", name="recr")
                        nc.vector.tensor_copy(recr[:], recf[:])
                        return recr

                    def emit_daos():
                        daos = []
                        for hh in range(2):
                            h = 2 * hp + hh
                            dao = nrmp.tile([128, 512], f32, tag="dao", name="dao")
                            # in the tail ACT is idle: split the copies so
                            # the final norm chain isn't DVE-serialized
                            if last_qg and hh == 0:
                                nc.scalar.copy(dao[:], ppv[h][:])
                            else:
                                nc.vector.tensor_copy(dao[:], ppv[h][:])
                            daos.append(dao)
                        return daos

                    if last_qg:
                        recr = emit_recs()
                        daos = emit_daos()
                    else:
                        # dao copies first: they gate ppv PSUM reuse for
                        # the next stream
                        daos = emit_daos()
                        recr = emit_recs()

                    def mk_norm(qg=qg, c2=hp, rec=recr, daos=daos):
                        def emit():
                            # two accumulating matmuls broadcast BOTH
                            # heads' 1/denom into one bc bank, then
                            # normalize into aot
                            bc = ps_bc.tile([128, 512], f32, tag="bc", name="bc")
                            for hh in range(2):
                                nc.tensor.matmul(
                                    bc[:],
                                    sel2[0:1, 128 * hh:128 * (hh + 1)],
                                    rec[0:1, 512 * hh:512 * (hh + 1)],
                                    start=(hh == 0), stop=(hh == 1))
                            for hh in range(2):
                                nc.vector.tensor_mul(
                                    aot[c2][64 * hh:64 * hh + 64,
                                            512 * qg:512 * (qg + 1)],
                                    daos[hh][64:128, :],
                                    bc[64 * hh:64 * hh + 64, :])
                        return emit
                    pending.append(mk_norm())
            # tail: the remaining normalize, then the last four
            # out-projection chunks out of wide ps_s tiles; split the
            # evacuation across ACT + DVE and the writeback DMAs across
            # the sync + gpsimd queues.
            while pending:
                emit_pending_one()
            for i in range(4):
                sc = 12 + i
                ps2 = ps_s.tile([128, 1024], f32, tag="ps", name="ps")
                for eg in range(2):
                    for c2 in range(2):
                        nc.tensor.matmul(
                            ps2[:, 512 * eg:512 * (eg + 1)],
                            aot[c2][:, 128 * sc:128 * (sc + 1)],
                            wov[:, c2, 512 * eg:512 * (eg + 1)],
                            start=(c2 == 0), stop=(c2 == 1))
                ysb = ysbp.tile([128, D], f16, tag="ysb", name="ysb")
                nc.scalar.copy(ysb[:, 0:512], ps2[:, 0:512])
                nc.vector.tensor_copy(ysb[:, 512:1024], ps2[:, 512:1024])
                for half in range(2):
                    sl = slice(512 * half, 512 * (half + 1))
                    eng = nc.sync if half == 0 else nc.gpsimd
                    eng.dma_start(
                        y[128 * sc:128 * (sc + 1), sl], ysb[:, sl])

    nc.compile()
    return nc


def _prep_inputs(x, token_positions, Wq, Wk, Wv, Wo):
    # even/odd interleave permutation within each head (for rotate-half RoPE)
    perm = np.concatenate([np.arange(0, DK, 2), np.arange(1, DK, 2)])

    pos = np.asarray(token_positions).astype(np.float32)
    angles = THETA ** (-np.arange(32, dtype=np.float32) / 32.0)
    ang = pos[:, None] * angles[None, :]          # [S, 32]
    cos32 = np.cos(ang).T.astype(np.float32)      # [32, S]
    sin32 = np.sin(ang).T.astype(np.float32)
    cos128 = np.concatenate([cos32, cos32, cos32, cos32], axis=0)
    sin128 = np.concatenate([-sin32, sin32, -sin32, sin32], axis=0)
    cos128 = np.ascontiguousarray(cos128).astype(np.float16)
    sin128 = np.ascontiguousarray(sin128).astype(np.float16)

    Wq = np.asarray(Wq, dtype=np.float32)
    Wk = np.asarray(Wk, dtype=np.float32)
    Wv = np.asarray(Wv, dtype=np.float32)
    Wo = np.asarray(Wo, dtype=np.float32)
    x = np.asarray(x, dtype=np.float32)

    f16 = np.float16

    def pack_w(wT):
        # [1024 d, 256 e] -> [128 p, 8 dc, 256 e]
        return np.ascontiguousarray(
            wT.reshape(8, 128, EL).transpose(1, 0, 2).reshape(128, 8 * EL)
        ).astype(f16)

    sel2 = np.zeros((1, 256), dtype=np.float32)
    sel2[0, 0:64] = 1.0
    sel2[0, 192:256] = 1.0

    in_maps = []
    for c in range(N_CORES):
        b = c // 4
        h0 = (c % 4) * HL
        esl = slice(h0 * DK, (h0 + HL) * DK)
        wq_h = Wq[esl].reshape(HL, DK, D)[:, perm].reshape(EL, D)
        wk_h = Wk[esl].reshape(HL, DK, D)[:, perm].reshape(EL, D)
        wv_h = Wv[esl]
        xT = x[b].T  # [1024 d, 2048 s]
        xt_p = np.ascontiguousarray(
            xT.reshape(8, 128, 4, 512).transpose(2, 1, 0, 3)).astype(f16)
        woT = Wo[:, esl].T  # [256 e, 1024 d_out]
        wo_p = np.ascontiguousarray(
            woT.reshape(2, 128, D).transpose(1, 0, 2).reshape(128, 2 * D)
        ).astype(f16)
        in_maps.append({
            "xt": xt_p,
            "wq": pack_w(wq_h.T),
            "wk": pack_w(wk_h.T),
            "wv": pack_w(wv_h.T),
            "wo": wo_p,
            "cosT": cos128,
            "sinT": sin128,
            "sel2": sel2,
        })
    return in_maps


def kernel(x, token_positions, Wq, Wk, Wv, Wo, _trace=False):
    from concourse.bass_utils import run_bass_kernel_spmd

    global _compiled
    if _compiled is None:
        _compiled = _build()
    in_maps = _prep_inputs(x, token_positions, Wq, Wk, Wv, Wo)
    res = run_bass_kernel_spmd(_compiled, in_maps, list(range(N_CORES)),
                               trace=_trace)
    parts = [res.results[c]["y"].astype(np.float64) for c in range(N_CORES)]
    out = np.empty((2, S, D), dtype=np.float32)
    out[0] = (parts[0] + parts[1] + parts[2] + parts[3]).astype(np.float32)
    out[1] = (parts[4] + parts[5] + parts[6] + parts[7]).astype(np.float32)
    if _trace:
        return out, res
    return out
